# revision 93
# baseline (speedup 1.0000x reference)
"""Trainium2 Bass kernel for nn_Encoder_Block (dense transformer encoder).

Data parallel across 8 NeuronCores (B=16 -> 2 batch elems per core), weights
replicated.  Reference structure (conv0 consumes RAW x+pos, not LN0 output):

  y0 = x+pos;  res0 = LN0(y0)
  y1 = relu(conv0(y0)) + res0
  y_{l+1} = relu(conv_l(LN_l(y_l))) + y_l        (l = 1..3)
  y5 = attn(LN4(y4)) + y4
  out = relu(fc(LN5(y5))) + y5

All LayerNorms here have uniform gain/bias, so LN(y) = a*y + beta with
runtime SCALARS a = g/sqrt(var+eps), beta = b - a*mu.  Since conv/attention/
fc are linear, the normalize pass is FOLDED into the matmuls:

  * every matmul consumes the UN-normalized residual y directly;
  * the post-PSUM pointwise op applies the scalars via the ACT engine's
    per-partition scale/bias operands: relu(a*psc + (beta*wsum + b2));
  * conv 'same' zero-padding stays exact by writing mu into the 3 pad
    columns per side (there a*mu + beta = 0, matching a zero-padded input);
  * the residual stream is stored OFFSET by -beta0 (LN0's offset); the
    offset is shift-invariant for variances, folds into downstream biases,
    and is added back once in the final residual add;
  * attention: q~ = a^2*qhat + (a*beta)*colsum(Wq'), k/v uncorrected (their
    offsets cancel in softmax / fold into the output bias beta*w_vo).

This removes every elementwise normalize pass and takes the LayerNorm stats
chain OFF the critical path: stats compute concurrently with the next
layer's matmuls (tiny cross-partition stats matmuls are interleaved into
the PE stream at points where their inputs are already available).

Key scheduling facts this kernel is tuned around:
  * x+pos is folded on the HOST and DMA'd STRAIGHT into the residual
    tiles (declared f32r), so conv0 starts as soon as x + w2-layer0 land
    (~4us); LN0 stats come from a full-T reduce (DVE) + Square (ACT).
  * input DMAs split across both HWDGE queues (SP: x + late weights,
    ACT: conv layers 0-1) so issue/transfer overlap.
  * residual adds for l>=1 are Pool tensor_tensor (lower latency than
    the old DMA-accumulate; Pool is otherwise idle).
  * the softmax denominator reciprocal is DVE reciprocal_approx_fast on
    a partition-shifted copy of the PSUM denominator row (custom DVE ops
    break on partition-shifted I/O, plain copies don't); the 1/d
    broadcast runs as an fp32 matmul (PE idle in the tail) so no f32r
    rounding hop is needed.  ACT does NO drain work for batch 0 (it is
    mid-exp-stream then); batch 1's drain copies ride on ACT (idle).
  * the 16 softmax exps are the only ACT work in the attention window;
    batch-1's PV uses lag=3 so its first PV (blocked on batch-0's psa
    drain) cannot stall the in-order PE queue; eT ring is 5 deep.
  * attn_out emits stage-ordered (both halves per stage) so the tail's
    DVE/ACT queues never serialize h1 behind h0's downstream ops.
  * out DMAs are emitted last (a dma_start burns ~1.3us of sequencer
    issue time) and split across SP+ACT queues for parallel issue.
  * the tile scheduler is readiness-greedy with COARSE per-engine sem
    counters: a wait rounds up to "everything scheduled before my dep on
    that engine has completed".  Emission order of latency-laggards
    (vt copy, b1 q/k copies, squares) around the first exps is load-
    bearing; several orderings here exist only to keep slow producers
    out of the exps' wait windows.
Engine split (Pool/gpsimd has no PSUM port and no accum_out, so):
  ACT:  affine+relu from PSUM (accum_out feeds the S1 recursion),
        softmax exp, LN0 square, b1 drain copies, tiny scalar chains.
  DVE:  LN0 reduce, squares (S2 accum), layer-0 add, q/k/av/avn copies,
        reciprocal, b1 fc-tail adds, PSUM-side tiny chain ops.
  Pool: residual adds l>=1, beta0-offset adds, b0 fc-tail adds,
        mu pad writes.
PE p-state: 2.4 GHz needs ~3us of busy ramp -> short junk-matmul warmup
while the first DMAs land.
"""

import sys

sys.path.insert(0, "/opt/trn_rl_repo")

import math

import numpy as np

import concourse.bass as bass
import concourse.tile as tile
from concourse import bacc, mybir
from concourse.bass_utils import run_bass_kernel_spmd

F32 = mybir.dt.float32
F32R = mybir.dt.float32r
AF = mybir.ActivationFunctionType
ALU = mybir.AluOpType

B, C, T = 16, 128, 1024
NCONV, KW = 4, 7
DK = C // 2
NCORES = 8
BPC = B // NCORES          # batch elems per core
EPS = 1e-5
NEL = float(C * T)
PADT = T + KW - 1          # 1030: per-batch padded row length in r tiles
NLN = NCONV + 2
HA = 520                   # x+pos chunk split: cols [0,520) cover conv h0

# cols blob layout (host-folded per-channel constants, one [C, NCOLS] DMA)
COL_WSUM = 0               # 4 cols: sum_{c,d} w2_l[c,d,o]
COL_B2 = COL_WSUM + NCONV  # 4 cols: folded conv bias
COL_U = COL_B2 + NCONV     # 1 col (rows 0:DK): colsum(Wq/sqrt(dk))
COL_WVO = COL_U + 1        # 1 col: colsum(Wv) @ Wo_eff
COL_CSF = COL_WVO + 1      # 1 col: colsum(fc_w.T)
COL_FCB = COL_CSF + 1      # 1 col: fc bias
COL_U2 = COL_FCB + 1       # 2 cols: [u, 0] (f32r-even pair for the ku mms)
NCOLS = COL_U2 + 2

# scal tile columns (per-LN runtime scalars, [C, 10])
SC_A = 0      # a = g*rsqrt(var+eps)
SC_B = 1      # beta = bc - a*mu_stored
SC_BIAS = 2   # folded bias col (conv: beta*wsum+b2; fc: beta*csf+fcb;
#               attn: beta*w_vo)
SC_NMU = 3    # -mu (col 4: -m2)
SC_NVAR = 5   # mu^2 - m2
SC_LNV = 6    # ln(var+eps)
SC_A2 = 7     # a^2 (LN4)
SC_AB = 8     # a*beta (LN4)
SC_ABU = 9    # (a*beta)*u  rows 0:DK (LN4)

DEBUG_TAPS = False


class _Bacc(bacc.Bacc):
    """Bacc with the activation-table choice pinned to the single set that
    covers every function used here, so exactly one table load is emitted."""

    _OURS = {AF.Ln, AF.Exp, AF.Square, AF.Copy, AF.Identity, AF.Relu}
    _KEEP = "natural_log_exp_and_others"

    def insert_act_table_loads(self):
        from concourse.bacc import _bass_rust, get_activation_tables
        has_activation = any(
            isinstance(i, mybir.InstActivation)
            for b in self.main_func.blocks
            for i in b.instructions
        )
        if not has_activation:
            return
        tables = [
            (nm, fs if nm == self._KEEP else (fs - self._OURS))
            for nm, fs in get_activation_tables(self.m.arch).items()
        ]
        _bass_rust.insert_act_table_loads(self, tables)


def _pos_encoding() -> np.ndarray:
    i = np.arange(C)
    exp = -((i - (i % 2)).astype(np.float32) / np.float32(C))
    freqs = (np.float32(10000.0) ** exp)[:, None].astype(np.float32)
    phases = ((i % 2).astype(np.float32) * np.float32(np.pi / 2))[:, None]
    pos = np.arange(T, dtype=np.float32)[None, :]
    return np.sin(pos * freqs + phases).astype(np.float32)


def _uniform_val(a: np.ndarray):
    v = a.flat[0]
    return float(v) if np.all(a == v) else None


def _build(gcs, bcs):
    """gcs/bcs: per-LN uniform gain/bias scalars (len NLN)."""
    assert all(g is not None and g > 0.0 for g in gcs), \
        "only uniform positive LN gains supported"
    assert all(b == 0.0 for b in bcs[1:NCONV]), \
        "mu-padding requires zero LN bias on conv-input norms"
    nc = _Bacc("TRN2", target_bir_lowering=False, debug=False,
               num_devices=NCORES)

    def tap(name, ap):
        if DEBUG_TAPS:
            d = nc.dram_tensor("dbg_" + name, list(ap.shape), F32,
                               kind="ExternalOutput").ap()
            nc.sync.dma_start(d, ap if ap.dtype == F32 else ap.bitcast(F32))

    def dram(name, shape, kind="ExternalInput"):
        return nc.dram_tensor(name, shape, F32, kind=kind).ap()

    x_d = nc.dram_tensor("x", [BPC, C, T], F32R,
                         kind="ExternalInput").ap()
    w2_d = dram("w2", [C, NCONV * KW * C])
    wqkv_d = dram("wqkv", [C, 3 * DK])
    wo_d = dram("wo", [DK, C])
    fcw_d = dram("fcw", [C, C])
    cols_d = dram("cols", [C, NCOLS])
    out_d = dram("out", [BPC, C, T], kind="ExternalOutput")

    from contextlib import ExitStack

    with tile.TileContext(nc) as tc, ExitStack() as ctx:
        cst = ctx.enter_context(tc.tile_pool(name="cst", bufs=1))
        rp = ctx.enter_context(tc.tile_pool(name="rp", bufs=2))
        work = ctx.enter_context(tc.tile_pool(name="work", bufs=2))
        attp = ctx.enter_context(tc.tile_pool(name="attp", bufs=2))
        tiny = ctx.enter_context(tc.tile_pool(name="tiny", bufs=4))
        ps_main = ctx.enter_context(
            tc.tile_pool(name="ps_main", bufs=2, space="PSUM"))
        ps_attn = ctx.enter_context(
            tc.tile_pool(name="ps_attn", bufs=1, space="PSUM"))
        ps_sm = ctx.enter_context(
            tc.tile_pool(name="ps_sm", bufs=2, space="PSUM"))

        # ---- constants ----
        ones_col = cst.tile([C, 1], F32, tag="ones_col")
        nc.vector.memset(ones_col[:], 1.0)
        ones_row = cst.tile([1, C], F32, tag="ones_row")
        nc.vector.memset(ones_row[:], 1.0)
        ones8 = cst.tile([C, 8], F32, tag="ones8")
        nc.vector.memset(ones8[:], 1.0)
        ones_row_r = cst.tile([1, C], F32R, tag="ones_row_r")
        nc.scalar.copy(ones_row_r[:], ones_row[:])


        const_tiles: dict = {}

        def const_ap(val: float):
            """[C,1] fp32 SBUF constant (activation bias operand)."""
            if val == 0.0:
                return 0.0   # pre-registered const AP
            if val not in const_tiles:
                t = cst.tile([C, 1], F32, tag=f"cst{len(const_tiles)}",
                             name=f"cst{len(const_tiles)}")
                nc.vector.memset(t[:], val)
                const_tiles[val] = t
            return const_tiles[val][:]

        # PE p-state warmup fodder: zeros, 512-wide
        junk = cst.tile([C, 512], F32, tag="junk")
        nc.gpsimd.memset(junk[:], 0.0)

        # ---- SBUF weight/const tiles ----
        w2_sb = cst.tile([C, NCONV * KW * C], F32R, tag="w2")
        wqkv_sb = cst.tile([C, 3 * DK], F32R, tag="wqkv")
        cols_sb = cst.tile([C, NCOLS], F32, tag="cols")
        wo_sb = cst.tile([DK, C], F32R, tag="wo")
        fcw_sb = cst.tile([C, C], F32R, tag="fcw")
        LW = KW * C

        # persistent beta0 (LN0 offset), added back in the final residual add
        beta0 = cst.tile([C, 1], F32, tag="beta0")
        # beta0 broadcast to a half-tile, so the final add can be two plain
        # Pool tensor_tensors (Pool has no scalar-ptr op)
        b0b = cst.tile([C, 512], F32, tag="b0b")
        # [u, 0] pair re-rounded to f32r (ku matmul moving operand)
        u2_sb = cst.tile([C, 2], F32R, tag="u2")

        # ---- residual tiles: one per batch elem, updated IN PLACE ----
        rtb = [rp.tile([C, PADT], F32R, tag=f"r{b}", name=f"r{b}")
               for b in range(BPC)]

        def ip(b, off=0, n=T):
            return rtb[b][:, 3 + off: 3 + off + n]

        def ipf(b, off=0, n=T):
            return ip(b, off, n).bitcast(F32)

        # zero pads for conv0 (true zero padding); layers 1-3 overwrite
        # these with their mu
        for b in range(BPC):
            nc.scalar.mul(rtb[b][:, 0:3], ones8[:, 0:3], 0.0)
            nc.scalar.mul(rtb[b][:, 3 + T:PADT], ones8[:, 0:3], 0.0)

        # ---- DMAs.  x (= input + pos, folded on host) lands STRAIGHT in
        # the residual tiles; SP carries x + late weights, ACT carries the
        # first two conv layers so both queues issue/transfer in parallel.
        nc.sync.dma_start(ip(0), x_d[0])
        nc.scalar.dma_start(w2_sb[:, 0:4 * C],
                            w2_d[:, 0:4 * C].bitcast(F32R))
        nc.sync.dma_start(ip(1), x_d[1])
        nc.scalar.dma_start(w2_sb[:, 4 * C:LW],
                            w2_d[:, 4 * C:LW].bitcast(F32R))
        nc.scalar.dma_start(w2_sb[:, LW:LW + 4 * C],
                            w2_d[:, LW:LW + 4 * C].bitcast(F32R))
        nc.scalar.dma_start(w2_sb[:, LW + 4 * C:2 * LW],
                            w2_d[:, LW + 4 * C:2 * LW].bitcast(F32R))
        nc.sync.dma_start(wqkv_sb[:], wqkv_d.bitcast(F32R))
        nc.sync.dma_start(cols_sb[:], cols_d[:])
        hwc = LW // 2
        for i in range(4, 8):
            nc.sync.dma_start(w2_sb[:, i * hwc:(i + 1) * hwc],
                              w2_d[:, i * hwc:(i + 1) * hwc].bitcast(F32R))
        nc.sync.dma_start(wo_sb[:], wo_d.bitcast(F32R))
        nc.sync.dma_start(fcw_sb[:], fcw_d.bitcast(F32R))

        nc.scalar.copy(u2_sb[:], cols_sb[:, COL_U2:COL_U2 + 2])

        # PE warmup: keeps the p-state busy-stretch alive until real matmuls
        for i in range(5):
            psj = ps_sm.tile([C, 512], F32, tag="sm", name=f"psj{i}")
            nc.tensor.matmul(psj[:], junk[:, 0:C].bitcast(F32R),
                             junk[:].bitcast(F32R), start=True, stop=True)

        # ---- per-(ln, b) stats state ----
        # st [C,4] cols: [h0-sum, h1-sum, S2_h0, S2_h1]
        #   direct2:  cols 0/1 = S1 half-accums (DVE adds / x+pos chunks)
        #   recurse2: cols 0/1 = A half-accums from the ACT affine
        #             (S1_l = A_h0 + A_h1 + S1_{l-1})
        # cols 2/3 = square-pass half accums (DVE).
        # The reduction is split in two stages: stage A (cols 0:2) gives the
        # MEAN early — the mu pad writes depend only on it, so the next
        # conv's taps can run in natural order with no pad stall; stage B
        # (cols 2:4) gives the variance -> a/beta/bias for the affine.
        # ssum [1,4]: col1 = S1 (stage A), col2 = S2 (stage B).
        class LNB:
            pass

        def recursive(ln, b):
            return not (ln <= 1 or ln == NLN - 1)

        lnbs = [[LNB() for _ in range(BPC)] for _ in range(NLN)]
        for ln in range(NLN):
            for b in range(BPC):
                s = lnbs[ln][b]
                s.st = tiny.tile([C, 4], F32, tag="st", name=f"st{ln}_{b}")
                s.scal = tiny.tile([C, 10], F32, tag="scal",
                                   name=f"scal{ln}_{b}")
                s.ssum = tiny.tile([1, 4], F32, tag="ssum",
                                   name=f"ssum{ln}_{b}")

        def emit_A1(ln, b):
            """Stage A part 1: reduce the S1/A half-accums (PE + DVE).
            ln==0 keeps S1 in a single st column (full-T reduce)."""
            s = lnbs[ln][b]
            s.pssa = ps_sm.tile([1, 4], F32, tag="sm", name=f"pssa{ln}{b}")
            if ln == 0:
                nc.tensor.matmul(s.pssa[:, 0:1], ones_col[:], s.st[:, 0:1],
                                 start=True, stop=True)
                nc.vector.tensor_copy(s.ssum[:, 1:2], s.pssa[:, 0:1])
                return
            nc.tensor.matmul(s.pssa[:, 0:2], ones_col[:], s.st[:, 0:2],
                             start=True, stop=True)
            nc.vector.tensor_copy(s.ssum[:, 0:2], s.pssa[:, 0:2])
            nc.vector.tensor_tensor(s.ssum[:, 1:2], s.ssum[:, 0:1],
                                    s.ssum[:, 1:2], ALU.add)
            if recursive(ln, b):
                prev = lnbs[ln - 1][b]
                nc.vector.tensor_tensor(s.ssum[:, 1:2], s.ssum[:, 1:2],
                                        prev.ssum[:, 1:2], ALU.add)

        def emit_A2(ln, b):
            """Stage A part 2: broadcast -mu; write the mu pads."""
            s = lnbs[ln][b]
            s.psba = ps_sm.tile([C, 2], F32, tag="sm", name=f"psba{ln}{b}")
            nc.tensor.matmul(s.psba[:, 0:1], ones_row[:], s.ssum[:, 1:2],
                             start=True, stop=True)
            nc.vector.tensor_scalar_mul(s.scal[:, SC_NMU:SC_NMU + 1],
                                        s.psba[:, 0:1], -1.0 / NEL)
            if 1 <= ln <= NCONV - 1:
                for ap in (rtb[b][:, 0:3], rtb[b][:, 3 + T:PADT]):
                    nc.vector.tensor_scalar(ap, ones8[:, 0:3],
                                            s.scal[:, SC_NMU:SC_NMU + 1],
                                            -1.0, op0=ALU.mult,
                                            op1=ALU.mult)

        def emit_B1(ln, b):
            """Stage B part 1: reduce the square half-accums (single col
            for ln==0)."""
            s = lnbs[ln][b]
            s.pssb = ps_sm.tile([1, 4], F32, tag="sm", name=f"pssb{ln}{b}")
            if ln == 0:
                nc.tensor.matmul(s.pssb[:, 0:1], ones_col[:], s.st[:, 2:3],
                                 start=True, stop=True)
                nc.vector.tensor_copy(s.ssum[:, 2:3], s.pssb[:, 0:1])
                return
            nc.tensor.matmul(s.pssb[:, 0:2], ones_col[:], s.st[:, 2:4],
                             start=True, stop=True)
            nc.vector.tensor_copy(s.ssum[:, 2:4], s.pssb[:, 0:2])
            nc.vector.tensor_tensor(s.ssum[:, 2:3], s.ssum[:, 2:3],
                                    s.ssum[:, 3:4], ALU.add)
        def emit_B2(ln, b):
            """Stage B part 2: -m2 broadcast; a/beta/bias scalar chain."""
            s = lnbs[ln][b]
            sc = s.scal
            s.psbb = ps_sm.tile([C, 2], F32, tag="sm", name=f"psbb{ln}{b}")
            nc.tensor.matmul(s.psbb[:, 0:1], ones_row[:], s.ssum[:, 2:3],
                             start=True, stop=True)
            nc.vector.tensor_scalar_mul(sc[:, SC_NMU + 1:SC_NMU + 2],
                                        s.psbb[:, 0:1], -1.0 / NEL)
            nc.vector.scalar_tensor_tensor(
                sc[:, SC_NVAR:SC_NVAR + 1], sc[:, SC_NMU:SC_NMU + 1],
                sc[:, SC_NMU:SC_NMU + 1], sc[:, SC_NMU + 1:SC_NMU + 2],
                op0=ALU.mult, op1=ALU.add)
            nc.scalar.activation(sc[:, SC_LNV:SC_LNV + 1],
                                 sc[:, SC_NVAR:SC_NVAR + 1], AF.Ln,
                                 scale=-1.0, bias=const_ap(EPS))
            nc.scalar.activation(sc[:, SC_A:SC_A + 1],
                                 sc[:, SC_LNV:SC_LNV + 1], AF.Exp,
                                 scale=-0.5,
                                 bias=const_ap(math.log(gcs[ln])))
            nc.vector.tensor_scalar(
                sc[:, SC_B:SC_B + 1], sc[:, SC_A:SC_A + 1],
                sc[:, SC_NMU:SC_NMU + 1], float(bcs[ln]),
                op0=ALU.mult, op1=ALU.add)
            if 1 <= ln <= NCONV - 1:      # conv-input LN: folded bias col
                nc.vector.scalar_tensor_tensor(
                    sc[:, SC_BIAS:SC_BIAS + 1],
                    cols_sb[:, COL_WSUM + ln:COL_WSUM + ln + 1],
                    sc[:, SC_B:SC_B + 1],
                    cols_sb[:, COL_B2 + ln:COL_B2 + ln + 1],
                    op0=ALU.mult, op1=ALU.add)
            elif ln == NCONV:             # attention input LN
                nc.scalar.activation(
                    sc[:, SC_A2:SC_A2 + 1], sc[:, SC_LNV:SC_LNV + 1],
                    AF.Exp, scale=-1.0,
                    bias=const_ap(2.0 * math.log(gcs[ln])))
                nc.vector.tensor_tensor(sc[:, SC_AB:SC_AB + 1],
                                        sc[:, SC_A:SC_A + 1],
                                        sc[:, SC_B:SC_B + 1], ALU.mult)
                nc.vector.tensor_tensor(sc[:, SC_BIAS:SC_BIAS + 1],
                                        cols_sb[:, COL_WVO:COL_WVO + 1],
                                        sc[:, SC_B:SC_B + 1], ALU.mult)
            elif ln == NCONV + 1:         # fc input LN
                nc.vector.scalar_tensor_tensor(
                    sc[:, SC_BIAS:SC_BIAS + 1],
                    cols_sb[:, COL_CSF:COL_CSF + 1],
                    sc[:, SC_B:SC_B + 1],
                    cols_sb[:, COL_FCB:COL_FCB + 1],
                    op0=ALU.mult, op1=ALU.add)
            if ln == 0:
                nc.vector.tensor_copy(beta0[:], sc[:, SC_B:SC_B + 1])
                nc.vector.tensor_scalar(b0b[:], junk[:], beta0[:],
                                        None, op0=ALU.add)

        def emit_sq(ln, b, h):
            """Half-T square pass of the current residual, accum -> st col
            2+h.  DVE normally; ACT for LN5-b1 (the tail, where DVE is
            the critical engine and ACT is idle after the exps)."""
            s = lnbs[ln][b]
            src = ipf(b, h * 512, 512)
            dump = work.tile([C, T], F32, tag="sqd", name=f"sqd{ln}_{b}")
            if (ln == NCONV + 1 and b == 1) or \
                    (1 <= ln <= NCONV - 1 and h == 0):
                # ACT square: LN5-b1 (tail, DVE critical) and the conv
                # LNs' h0 (frees DVE for the h1 residual adds)
                nc.scalar.activation(dump[:, 0:512], src, AF.Square,
                                     accum_out=s.st[:, 2 + h:3 + h])
            else:
                nc.vector.scalar_tensor_tensor(
                    dump[:, 0:512], src, 1.0, src, op0=ALU.mult,
                    op1=ALU.mult, accum_out=s.st[:, 2 + h:3 + h])

        # ================= LN0 stats (y0 = x+pos arrives via DMA) ========
        def emit_ln0(b):
            """Full-T S1 reduce (DVE) + full-T square accum (ACT) into the
            single-column st slots the ln==0 A1/B1 stages expect."""
            s0 = lnbs[0][b]
            nc.vector.tensor_reduce(s0.st[:, 0:1], ipf(b),
                                    mybir.AxisListType.X, ALU.add)
            dump = work.tile([C, T], F32, tag="sqd", name=f"sqd0_{b}")
            nc.scalar.activation(dump[:], ipf(b), AF.Square,
                                 accum_out=s0.st[:, 2:3])
            if b == 0:
                tap("y0", ipf(0))

        # ================= conv layers =================
        # Natural tap order (pads are pre-written by the PREVIOUS layer's
        # stage-A, which needs only the mean): the h0 PSUM group closes at
        # ~1.5us so the affine/add/square chain starts mid-conv.  Exception:
        # layer 1's pads depend on the layer-0 DVE adds and are late, so
        # conv(1,*) uses interior-first order.
        # Job slots (tiny stats matmuls interleaved where inputs are ready):
        #   conv(l,0):  S1 B1(l,b0) | S2 B2(l,b0) | S3 A1(l,b1) | S4 A2(l,b1)
        #   conv(l,1):  S1 B1(l,b1) | S2 B2(l,b1) | S3 A1(l+1,b0) | S4 A2
        #   conv(0,b):  A(0,b) in S1/S2, B(0,b) in S3/S4
        #   (A(1,b0) rides at the end of conv(0,1).)
        def emit_pointwise(l, b, psc):
            """Per-half affine-relu (ACT) + in-place residual add.
            l=0: DVE scalar_tensor_tensor with direct S1 accums.  l>=1:
            Pool tensor_tensor add; S1 via the affine's half accums."""
            s = lnbs[l][b]
            nxt = lnbs[l + 1][b]
            relu_t = s.relu_t
            for h in range(2):
                sl = slice(h * 512, (h + 1) * 512)
                if l == 0:
                    nc.scalar.activation(relu_t[:, sl], psc[:, sl], AF.Relu,
                                         bias=cols_sb[:, COL_B2:COL_B2 + 1],
                                         scale=1.0)
                    nc.vector.scalar_tensor_tensor(
                        ip(b, h * 512, 512), ipf(b, h * 512, 512),
                        s.scal[:, SC_A:SC_A + 1], relu_t[:, sl],
                        op0=ALU.mult, op1=ALU.add,
                        accum_out=nxt.st[:, h:h + 1])
                else:
                    nc.scalar.activation(relu_t[:, sl], psc[:, sl], AF.Relu,
                                         bias=s.scal[:, SC_BIAS:SC_BIAS + 1],
                                         scale=s.scal[:, SC_A:SC_A + 1],
                                         accum_out=nxt.st[:, h:h + 1])
                    # h0 add on Pool; h1 add on DVE (it gates the next
                    # layer's first taps, and DVE has slack now that the
                    # h0 squares run on ACT).  Exception: the LAST conv's
                    # b1 adds stay on Pool — a DVE add there lands inside
                    # the first exp's coarse DVE wait window
                    if h == 0 or (l == NCONV - 1 and b == 1):
                        nc.gpsimd.tensor_tensor(ip(b, h * 512, 512),
                                                ipf(b, h * 512, 512),
                                                relu_t[:, sl], ALU.add)
                    else:
                        nc.vector.tensor_tensor(ip(b, h * 512, 512),
                                                ipf(b, h * 512, 512),
                                                relu_t[:, sl], ALU.add)
                if not (l == NCONV - 1 and b == 1):
                    # sq(4, b1) is emitted inside the attention block so it
                    # does not clog the DVE queue at the attention start
                    emit_sq(l + 1, b, h)

        def conv_taps(l, b, h, psc, dlist, start_d, stop_d):
            for d in dlist:
                nc.tensor.matmul(
                    psc[:, h * 512:(h + 1) * 512],
                    w2_sb[:, (l * KW + d) * C:(l * KW + d + 1) * C],
                    rtb[b][:, h * 512 + d: h * 512 + d + 512],
                    start=(d == start_d), stop=(d == stop_d))

        def emit_conv(l, b, jobs):
            """jobs: 4 callables interleaved between tap groups."""
            s = lnbs[l][b]
            s.relu_t = work.tile([C, T], F32, tag="relu",
                                 name=f"relu{l}_{b}")
            psc = ps_main.tile([C, T], F32, tag="main", name=f"psc{l}_{b}")
            if l == 0:
                groups = [(0, [0, 1, 2, 3], 0, 6), None,
                          (0, [4, 5, 6], 0, 6), None, (1, [0, 1], 0, 6),
                          None, (1, [2, 3], 0, 6), None,
                          (1, [4, 5, 6], 0, 6)]
            else:
                # h0 interior-first: +0.9us of slack for this layer's mu
                # pads (written by the previous conv's stage-A job) at the
                # same h0-group close time
                groups = [(0, [3, 4, 5, 6], 3, 2), None,
                          (0, [0, 1, 2], 3, 2), None, (1, [0, 1], 0, 6),
                          None, (1, [2, 3], 0, 6), None,
                          (1, [4, 5, 6], 0, 6)]
            ji = iter(jobs)
            for g in groups:
                if g is None:
                    j = next(ji, None)
                    if j:
                        j()
                else:
                    conv_taps(l, b, g[0], psc, g[1], g[2], g[3])
            for j in ji:
                if j:
                    j()
            emit_pointwise(l, b, psc)
            if b == 0:
                tap(f"y{l + 1}", ipf(0))

        def stats_jobs(*specs):
            """specs: (stage_fn..., ln, b) callables list builder."""
            return [
                (lambda f=f, ln=ln, b=b: f(ln, b))
                for (f, ln, b) in specs
            ]

        # ================= attention =================
        # The LN4 q-side correction is folded into the EXP instead of the q
        # copy:  eT = exp(a^2 * (khat^T qhat) + a*beta * (khat^T u)[key]),
        # via the ACT scale (a^2, broadcast col) and bias (per-key col from
        # 8 tiny khat^T u matmuls).  The q/k PSUM->SBUF copies are then
        # plain tensor_copies with no stats dependency.  ACT does ONLY the
        # 16 exps in this phase; drains/normalize run on DVE/Pool.
        def attn_qk_mms(b):
            """k then q matmuls + PSUM->SBUF copies (k first: psku needs
            it).  b0 rides as a job inside conv(3,1); b1 inside batch-0's
            scores stream (ps_sm halves keep it off the scores ring)."""
            s4 = lnbs[NCONV][b]
            qk = attp.tile([DK, 2 * T], F32R, tag="qk", name=f"qk{b}")
            s4.qk = qk
            if b == 0:
                psq = ps_main.tile([C, T], F32, tag="main", name="psq0")
                psk = ps_attn.tile([C, T], F32, tag="attn", name="psk0")
                for h in range(2):
                    nc.tensor.matmul(psk[0:DK, h * 512:(h + 1) * 512],
                                     wqkv_sb[:, DK:2 * DK],
                                     ip(b, h * 512, 512),
                                     start=True, stop=True)
                for h in range(2):
                    nc.tensor.matmul(psq[0:DK, h * 512:(h + 1) * 512],
                                     wqkv_sb[:, 0:DK],
                                     ip(b, h * 512, 512),
                                     start=True, stop=True)
                # k copies first: the psku->kub chain must be READY
                # before the scheduler picks the vt copy / squares, or
                # the first exp's coarse sem wait inherits their lateness
                for h in range(2):
                    sl = slice(h * 512, (h + 1) * 512)
                    nc.vector.tensor_copy(
                        qk[:, T + h * 512:T + (h + 1) * 512],
                        psk[0:DK, sl])
                for h in range(2):
                    sl = slice(h * 512, (h + 1) * 512)
                    nc.vector.tensor_copy(qk[:, sl], psq[0:DK, sl])
            else:
                for h in range(2):
                    pskh = ps_sm.tile([C, 512], F32, tag="sm",
                                      name=f"psk{b}_{h}")
                    nc.tensor.matmul(pskh[0:DK, :], wqkv_sb[:, DK:2 * DK],
                                     ip(b, h * 512, 512),
                                     start=True, stop=True)
                    nc.vector.tensor_copy(
                        qk[:, T + h * 512:T + (h + 1) * 512],
                        pskh[0:DK, :])
                for h in range(2):
                    psqh = ps_sm.tile([C, 512], F32, tag="sm",
                                      name=f"psq{b}_{h}")
                    nc.tensor.matmul(psqh[0:DK, :], wqkv_sb[:, 0:DK],
                                     ip(b, h * 512, 512),
                                     start=True, stop=True)
                    nc.vector.tensor_copy(qk[:, h * 512:(h + 1) * 512],
                                          psqh[0:DK, :])
            if b == 0:
                tap("qk0", qk[:])

        def attn_ku(b):
            """per-key khat^T u (exp bias cols) + kub scaling (DVE)."""
            s4 = lnbs[NCONV][b]
            psku = ps_sm.tile([C, 16], F32, tag="sm", name=f"psku{b}")
            for j in range(8):
                nc.tensor.matmul(
                    psku[:, 2 * j:2 * j + 2],
                    s4.qk[:, T + j * C:T + (j + 1) * C],
                    u2_sb[0:DK, :],
                    start=True, stop=True)
            s4.kub = tiny.tile([C, 8], F32, tag="kub", name=f"kub{b}")
            nc.vector.tensor_scalar(
                s4.kub[:].rearrange("p (j o) -> p j o", o=1),
                psku[:].rearrange("p (j two) -> p j two",
                                  two=2)[:, :, 0:1],
                s4.scal[:, SC_AB:SC_AB + 1], None, op0=ALU.mult)

        def attn_v(b):
            """v matmuls + vt assembly (copies on DVE/Pool, not ACT)."""
            s4 = lnbs[NCONV][b]
            psv = ps_sm.tile([C, 512], F32, tag="sm", name=f"psv{b}")
            for j in range(8):
                nc.tensor.matmul(psv[:, j * DK:(j + 1) * DK],
                                 ip(b, j * C, C),
                                 wqkv_sb[:, 2 * DK:3 * DK],
                                 start=True, stop=True)
            vt = attp.tile([C, 8, DK + 1], F32R, tag="vt", name=f"vt{b}")
            nc.gpsimd.tensor_copy(
                vt[:, :, DK:DK + 1],
                ones8[:].rearrange("p (j o) -> p j o", o=1))
            nc.vector.tensor_copy(
                vt[:, :, 0:DK],
                psv[:].rearrange("p (j k) -> p j k", k=DK))
            s4.vt = vt

        def attn_scores_pv(b, hooks, lag=1):
            """scores -> exp -> PV pipeline, j-outer with a deep eT ring.
            hooks: {j: callable} run after scores(j).  lag: how many j's
            the PV trails the scores stream — batch 1 uses a larger lag so
            its first PV (which must wait for batch 0's psa drain) sits
            late enough in the in-order PE queue not to stall scores."""
            s4 = lnbs[NCONV][b]
            qk = s4.qk
            psa = ps_attn.tile([C, T], F32, tag="attn", name=f"psa{b}")
            eTs = [None] * 8

            def scores(j):
                pss = ps_main.tile([C, T], F32, tag="main",
                                   name=f"pss{b}_{j}")
                for h in range(2):
                    nc.tensor.matmul(pss[:, h * 512:(h + 1) * 512],
                                     qk[:, T + j * C:T + (j + 1) * C],
                                     qk[:, h * 512:h * 512 + 512],
                                     start=True, stop=True)
                return pss

            def exp(j, pss):
                eTs[j] = attp.tile([C, T], F32R, tag="eT", bufs=6,
                                   name=f"eT{b}_{j}")
                nc.scalar.activation(eTs[j][:], pss[:], AF.Exp,
                                     scale=s4.scal[:, SC_A2:SC_A2 + 1],
                                     bias=s4.kub[:, j:j + 1])

            def pv(j):
                vt = s4.vt    # produced by attn_v, possibly via a hook
                for h in range(2):
                    nc.tensor.matmul(
                        psa[0:DK + 1, h * 512:(h + 1) * 512], vt[:, j, :],
                        eTs[j][:, h * 512:(h + 1) * 512],
                        start=(j == 0), stop=(j == 7))

            for j in range(8):
                pss = scores(j)
                if j in hooks:
                    hooks[j]()
                exp(j, pss)
                if j >= lag:
                    pv(j - lag)
            for j in range(8 - lag, 8):
                pv(j)
            s4.psa = psa
            if b == 0:
                tap("eT0", eTs[0][:])

        def attn_drain(b):
            """Drain psa: av copy + denominator row (partition-shifted
            PSUM read), 1/denom via the DVE fast-reciprocal, rounded to
            f32r on Pool.  No ACT work."""
            s4 = lnbs[NCONV][b]
            av = attp.tile([DK, T], F32R, tag="av", name=f"av{b}")
            d0 = tiny.tile([1, T], F32, tag="lnd", name=f"d0{b}")
            rr = tiny.tile([1, T], F32, tag="rr0", name=f"rr{b}")
            # denominator first (it heads the psr->avn chain), av after.
            # b1 (the tail) drains on ACT, which is idle after the exps —
            # DVE is the tail's critical engine.  rr stays fp32: the psr
            # broadcast runs as an fp32 matmul (PE is idle in the tail).
            cp = nc.scalar.copy if b == 1 else nc.vector.tensor_copy
            for h in range(2):
                sl = slice(h * 512, (h + 1) * 512)
                cp(d0[:, sl], s4.psa[DK:DK + 1, sl])
                nc.vector.reciprocal_approx_fast(rr[:, sl], d0[:, sl])
            for h in range(2):
                sl = slice(h * 512, (h + 1) * 512)
                cp(av[:, sl], s4.psa[0:DK, sl])
            s4.av, s4.rr = av, rr

        def attn_out(b):
            """1/denom broadcast (PE) -> normalize (DVE) -> Wo -> affine
            -> res add.  PSUM from ps_sm halves so batch-0's tail never
            touches the scores ring batch-1 is rotating through."""
            s4 = lnbs[NCONV][b]
            s5 = lnbs[NCONV + 1][b]
            avn = attp.tile([DK, T], F32R, tag="avn", name=f"avn{b}")
            att = work.tile([C, T], F32, tag="relu", name=f"att{b}")
            psos = [None, None]
            # stage-ordered emission: both halves of each stage together,
            # so the DVE/ACT queues never serialize h1 behind h0's
            # downstream ops
            for h in range(2):
                sl = slice(h * 512, (h + 1) * 512)
                psr = ps_sm.tile([C, 512], F32, tag="sm",
                                 name=f"psr{b}_{h}")
                nc.tensor.matmul(psr[0:DK, :], ones_row[:, 0:DK],
                                 s4.rr[:, sl], start=True, stop=True)
                nc.vector.tensor_tensor(avn[:, sl], s4.av[:, sl],
                                        psr[0:DK, :], ALU.mult)
            for h in range(2):
                sl = slice(h * 512, (h + 1) * 512)
                psos[h] = ps_sm.tile([C, 512], F32, tag="sm",
                                     name=f"pso{b}_{h}")
                nc.tensor.matmul(psos[h][:], wo_sb[:], avn[:, sl],
                                 start=True, stop=True)
            for h in range(2):
                sl = slice(h * 512, (h + 1) * 512)
                if b == 0:
                    nc.vector.tensor_scalar(
                        att[:, sl], psos[h][:],
                        s4.scal[:, SC_A:SC_A + 1],
                        s4.scal[:, SC_BIAS:SC_BIAS + 1], op0=ALU.mult,
                        op1=ALU.add)
                else:
                    nc.scalar.activation(
                        att[:, sl], psos[h][:], AF.Identity,
                        bias=s4.scal[:, SC_BIAS:SC_BIAS + 1],
                        scale=s4.scal[:, SC_A:SC_A + 1])
            for h in range(2):
                sl = slice(h * 512, (h + 1) * 512)
                nc.vector.scalar_tensor_tensor(
                    ip(b, h * 512, 512), att[:, sl], 1.0,
                    ipf(b, h * 512, 512), op0=ALU.mult, op1=ALU.add,
                    accum_out=s5.st[:, h:h + 1])
                if b == 1:
                    # batch-1 tail: issue this half's FC matmul right away
                    nc.tensor.matmul(psfs[b][:, sl], fcw_sb[:],
                                     ip(b, h * 512, 512),
                                     start=True, stop=True)
            # squares AFTER both halves' affines: they must not sit
            # between the two att ops in the in-order ACT/DVE queues
            emit_sq(NCONV + 1, b, 0)
            emit_sq(NCONV + 1, b, 1)
            if b == 0:
                tap("y5", ipf(0))

        # ---- fc pieces ----
        relu5s = [None, None]
        outs = [None, None]
        psfs = [None, None]

        def fc_alloc(b):
            relu5s[b] = work.tile([C, T], F32, tag="relu",
                                  name=f"relu5_{b}")
            outs[b] = work.tile([C, T], F32, tag="out", name=f"out{b}")
            psfs[b] = ps_main.tile([C, T], F32, tag="main", name=f"psf{b}")

        def fc_mms(b):
            fc_alloc(b)
            for h in range(2):
                nc.tensor.matmul(psfs[b][:, h * 512:(h + 1) * 512],
                                 fcw_sb[:], ip(b, h * 512, 512),
                                 start=True, stop=True)

        def fc_tail(b):
            s5 = lnbs[NCONV + 1][b]
            # batch 1 (the critical tail) works in finer chunks so the
            # last DMA's chain starts earlier
            chunks = [(0, 512), (512, 512)]
            for o, n in chunks:
                # out = (y5 + beta0) early on Pool: no relu dependency
                nc.gpsimd.tensor_tensor(outs[b][:, o:o + n],
                                        ipf(b, o, n), b0b[:, 0:n],
                                        ALU.add)
            for o, n in chunks:
                sl = slice(o, o + n)
                nc.scalar.activation(relu5s[b][:, sl], psfs[b][:, sl],
                                     AF.Relu,
                                     bias=s5.scal[:, SC_BIAS:SC_BIAS + 1],
                                     scale=s5.scal[:, SC_A:SC_A + 1])
                # += relu5: DVE for the latency-critical batch-1 tail
                # (DVE is free there); Pool for batch 0 (DVE is mid
                # batch-1 chain then)
                if b == 1:
                    nc.vector.tensor_tensor(outs[b][:, sl],
                                            outs[b][:, sl],
                                            relu5s[b][:, sl], ALU.add)
                else:
                    nc.gpsimd.tensor_tensor(outs[b][:, sl],
                                            outs[b][:, sl],
                                            relu5s[b][:, sl], ALU.add)
            # DMAs last, so their ~1.3us HWDGE issue never sits between
            # compute ops in the ACT sequencer; b0 uses the idle SP
            # queue, b1 splits across both queues for parallel issue
            for k, (o, n) in enumerate(chunks):
                sl = slice(o, o + n)
                if b == 1 and k < len(chunks) - 1:
                    nc.scalar.dma_start(out_d[b][:, sl], outs[b][:, sl])
                else:
                    nc.sync.dma_start(out_d[b][:, sl], outs[b][:, sl])

        # ====== orchestration ======
        emit_ln0(0)
        emit_ln0(1)
        emit_conv(0, 0, stats_jobs((emit_A1, 0, 0), (emit_A2, 0, 0),
                                   (emit_B1, 0, 0)) +
                  [lambda: (emit_B2(0, 0))])
        emit_conv(0, 1, stats_jobs((emit_A1, 0, 1), (emit_A2, 0, 1),
                                   (emit_B1, 0, 1)) +
                  [lambda: (emit_B2(0, 1), emit_A1(1, 0), emit_A2(1, 0))])
        for l in range(1, NCONV):
            emit_conv(l, 0, stats_jobs((emit_B1, l, 0)) +
                      [lambda l=l: (emit_B2(l, 0)),
                       lambda l=l: (emit_A1(l, 1)),
                       lambda l=l: (emit_A2(l, 1))])
            if l < NCONV - 1:
                emit_conv(l, 1, stats_jobs((emit_B1, l, 1)) +
                          [lambda l=l: (emit_B2(l, 1)),
                           lambda l=l: (emit_A1(l + 1, 0)),
                           lambda l=l: (emit_A2(l + 1, 0))])
            else:
                # last conv: LN4-b0 stage-B + batch-0's attention q/k
                # matmuls ride in the job slots / tail of the tap stream
                emit_conv(l, 1, stats_jobs((emit_B1, l, 1)) +
                          [lambda: (emit_B2(NCONV - 1, 1),
                                    emit_A1(NCONV, 0)),
                           lambda: (emit_A2(NCONV, 0),
                                    emit_B1(NCONV, 0)),
                           lambda: emit_B2(NCONV, 0),
                           lambda: attn_qk_mms(0)])

        # PE stream: b0 q/k rides inside conv(3,1); b0 ku/v hook into the
        # first scores | b0 scores+pv with b1 qkv+stats hooked in | b1
        # scores+pv with the b0 tail and LN5-b0 stats hooked in | b1 tail
        attn_ku(0)
        attn_v(0)
        attn_scores_pv(0, {
            1: lambda: emit_A1(NCONV, 1),
            2: lambda: (attn_qk_mms(1), emit_A2(NCONV, 1)),
            4: lambda: (emit_sq(NCONV, 1, 0), emit_sq(NCONV, 1, 1)),
            5: lambda: (emit_B1(NCONV, 1), emit_B2(NCONV, 1),
                        attn_ku(1)),
            6: lambda: attn_v(1),
        }, lag=2)
        attn_drain(0)
        attn_scores_pv(1, {
            1: lambda: attn_out(0),
            4: lambda: emit_A1(NCONV + 1, 0),
            5: lambda: emit_A2(NCONV + 1, 0),
            6: lambda: emit_B1(NCONV + 1, 0),
            7: lambda: emit_B2(NCONV + 1, 0),
        }, lag=3)
        fc_mms(0)
        attn_drain(1)
        fc_tail(0)
        fc_alloc(1)
        attn_out(1)
        emit_A1(NCONV + 1, 1)
        emit_A2(NCONV + 1, 1)
        emit_B1(NCONV + 1, 1)
        emit_B2(NCONV + 1, 1)
        fc_tail(1)

    nc.compile()
    return nc


_CACHE: dict = {}
LAST_RUN: dict = {}


def kernel(x, mask, dw_w, dw_b, pw_w, pw_b, norm0_g, norm0_b,
           norms_g, norms_b, norme_g, norme_b,
           Wq, Wk, Wv, Wo, fc_w, fc_b):
    x = np.asarray(x, dtype=np.float32)
    mask = np.asarray(mask, dtype=np.float32)
    assert np.all(mask == 1.0), "only all-ones mask supported"

    # ---- host-side folding ----
    w2 = np.empty((C, NCONV, KW, C), dtype=np.float32)
    for i in range(NCONV):
        pwT = np.asarray(pw_w[i], np.float32).T
        for d in range(KW):
            w2[:, i, d, :] = pwT * np.asarray(dw_w[i][:, d],
                                              np.float32)[:, None]
    wsum = w2.sum(axis=(0, 2))                            # [NCONV, C] (o)
    w2 = w2.reshape(C, NCONV * KW * C)
    b2 = np.stack([np.asarray(pw_w[i], np.float32)
                   @ np.asarray(dw_b[i], np.float32)
                   + np.asarray(pw_b[i], np.float32)
                   for i in range(NCONV)])                 # [NCONV, C]
    wq = np.asarray(Wq, np.float32) / math.sqrt(DK)
    wqkv = np.concatenate([wq, np.asarray(Wk, np.float32),
                           np.asarray(Wv, np.float32)], axis=1)
    wo = np.asarray(Wo, np.float32)
    wo_eff = np.ascontiguousarray(wo[:DK] + wo[DK:])
    fcw = np.ascontiguousarray(np.asarray(fc_w, np.float32).T)
    pos = _pos_encoding()

    cols = np.zeros((C, NCOLS), np.float32)
    cols[:, COL_WSUM:COL_WSUM + NCONV] = wsum.T
    cols[:, COL_B2:COL_B2 + NCONV] = b2.T
    cols[0:DK, COL_U] = wq.sum(axis=0)
    cols[0:DK, COL_U2] = wq.sum(axis=0)
    cols[:, COL_WVO] = np.asarray(Wv, np.float32).sum(axis=0) @ wo_eff
    cols[:, COL_CSF] = fcw.sum(axis=0)
    cols[:, COL_FCB] = np.asarray(fc_b, np.float32)

    gs = [norm0_g] + [norms_g[i] for i in range(NCONV)] + [norme_g]
    bs = [norm0_b] + [norms_b[i] for i in range(NCONV)] + [norme_b]
    gcs = tuple(_uniform_val(np.asarray(g, np.float32)) for g in gs)
    bcs = tuple(_uniform_val(np.asarray(bb, np.float32)) for bb in bs)
    assert all(g is not None for g in gcs), "non-uniform LN gain unsupported"
    assert all(b is not None for b in bcs), "non-uniform LN bias unsupported"

    key = (gcs, bcs)
    if key not in _CACHE:
        _CACHE[key] = _build(gcs, bcs)
    nc = _CACHE[key]

    xp = x + pos[None]          # positional encoding folded on the host
    base = {"w2": w2, "wqkv": wqkv, "wo": wo_eff, "fcw": fcw,
            "cols": cols}
    in_maps = []
    for c in range(NCORES):
        m = dict(base)
        m["x"] = np.ascontiguousarray(xp[c * BPC:(c + 1) * BPC])
        in_maps.append(m)

    LAST_RUN["nc"] = nc
    LAST_RUN["in_maps"] = in_maps

    res = run_bass_kernel_spmd(nc, in_maps, list(range(NCORES)))
    out = np.concatenate([r["out"] for r in res.results], axis=0)
    return out.astype(np.float32)

